# revision 13
# baseline (speedup 1.0000x reference)
"""Trainium2 Bass kernel for nn_Adj_layer (pairwise-diff conv stack + BN +
softmax + top-k masking), data-parallel over the batch axis on 8 NeuronCores.

Self-contained: hardcodes all shapes. Needs the concourse toolchain on the
python path (stock location /opt/trn_rl_repo inside the TRN2 container).
"""

import os
import sys

for _p in ("/opt/trn_rl_repo", os.path.expanduser("~/.axon_site/_ro/trn_rl_repo")):
    if os.path.isdir(_p) and _p not in sys.path:
        sys.path.insert(0, _p)

import numpy as np

import concourse.bacc as bacc
import concourse.bass as bass
import concourse.mybir as mybir
import concourse.tile as tile
from concourse.bass_utils import run_bass_kernel_spmd

F32 = mybir.dt.float32
AF = mybir.ActivationFunctionType
ALU = mybir.AluOpType

N_CORES = 8
B, V, D, H = 8, 201, 256, 128
NPIX = V * V                # 40401 pixels per batch element
NTOT = B * NPIX             # BN statistics population
K = 100                     # top-k
EPS = 1e-5
SLOPE = 0.01
CHUNK = 2 * V               # 402 pixels = 2 rows of the VxV map
NCHUNK = (NPIX + CHUNK - 1) // CHUNK   # 101 (last chunk = 1 row)
CIN = [D, 2 * H, 2 * H, H]  # per-block input channels
COUT = [2 * H, 2 * H, H, H]


def _build_nc(trace_scopes=False):
    nc = bacc.Bacc("TRN2", target_bir_lowering=False, num_devices=N_CORES)

    # ---- external I/O (per-core) ----
    xt_d = nc.dram_tensor("xt", [128, 2 * V], F32, kind="ExternalInput")
    w_d = [
        nc.dram_tensor("w0t", [128, 512], F32, kind="ExternalInput"),
        nc.dram_tensor("w1t", [128, 512], F32, kind="ExternalInput"),
        nc.dram_tensor("w2t", [128, 256], F32, kind="ExternalInput"),
        nc.dram_tensor("w3t", [128, 128], F32, kind="ExternalInput"),
    ]
    w4_d = nc.dram_tensor("w4t", [128, 1], F32, kind="ExternalInput")
    # per-block packed params: [g | be | b] each cout/128 columns
    p_d = [
        nc.dram_tensor(f"p{k}", [128, 3 * (COUT[k] // 128)], F32, kind="ExternalInput")
        for k in range(4)
    ]
    out_d = nc.dram_tensor("outb", [V, V], F32, kind="ExternalOutput")

    from contextlib import ExitStack
    with tile.TileContext(nc) as tc, ExitStack() as stack:
        dram = stack.enter_context(tc.tile_pool(name="dram", bufs=1, space="DRAM"))
        resid = stack.enter_context(tc.tile_pool(name="resid", bufs=1))
        psum = stack.enter_context(tc.tile_pool(name="psum", bufs=6, space="PSUM"))
        psum1 = stack.enter_context(tc.tile_pool(name="psum1", bufs=2, space="PSUM"))

        # internal DRAM: activation bounce buffers + logits
        ybuf = [[dram.tile([128, NPIX], F32, tag=f"y{k}_{ot}", name=f"y{k}_{ot}") for ot in range(2)]
                for k in range(2)]
        logits_d = dram.tile([V, V], F32, tag="logits", name="logits")
        ar_in = [dram.tile([128, 2 * (COUT[k] // 128)], F32, tag=f"arin{k}", name=f"arin{k}")
                 for k in range(4)]
        ar_out = [dram.tile([128, 2 * (COUT[k] // 128)], F32, tag=f"arout{k}", name=f"arout{k}")
                  for k in range(4)]

        # resident SBUF
        xT = resid.tile([128, 2 * V], F32, tag="xT", name="xT")
        wsb = [resid.tile([128, w_d[k].shape[1]], F32, tag=f"w{k}", name=f"wsb{k}") for k in range(4)]
        w4sb = resid.tile([128, 1], F32, tag="w4", name="w4sb")
        psb = [resid.tile([128, p_d[k].shape[1]], F32, tag=f"p{k}", name=f"psb{k}") for k in range(4)]
        bigbuf = resid.tile([128, NPIX], F32, tag="bigbuf", name="bigbuf")   # y2 then y3
        sumc = [resid.tile([128, NCHUNK], F32, tag=f"sumc{ot}", name=f"sumc{ot}") for ot in range(2)]
        sumsqc = [resid.tile([128, NCHUNK], F32, tag=f"sumsqc{ot}", name=f"sumsqc{ot}") for ot in range(2)]
        # per-block BN affine params
        s_sb = [resid.tile([128, COUT[k] // 128], F32, tag=f"s{k}", name=f"s_sb{k}") for k in range(4)]
        t_sb = [resid.tile([128, COUT[k] // 128], F32, tag=f"t{k}", name=f"t_sb{k}") for k in range(4)]

        nc.sync.dma_start(xT[:], xt_d[:])
        for k in range(4):
            nc.sync.dma_start(wsb[k][:], w_d[k][:])
            nc.sync.dma_start(psb[k][:], p_d[k][:])
        nc.sync.dma_start(w4sb[:], w4_d[:])

        def chunk_pixels(ch):
            n0 = ch * CHUNK
            return n0, min(CHUNK, NPIX - n0)

        def stats_and_store(k, ch, ps_tiles, store_fn, scr_pool):
            """Copy conv output (psum) to its destination and accumulate
            per-channel sum / sum-of-squares partials for chunk ch."""
            _, npx = chunk_pixels(ch)
            nt = COUT[k] // 128
            for ot in range(nt):
                dst = store_fn(ot)
                nc.scalar.activation(dst, ps_tiles[ot][:, :npx], AF.Copy,
                                     accum_out=sumc[ot][:, ch:ch + 1])
                scr = scr_pool.tile([128, CHUNK], F32, tag="scr", name="scr")
                nc.vector.scalar_tensor_tensor(
                    scr[:, :npx], dst, 1.0, dst,
                    op0=ALU.mult, op1=ALU.mult,
                    accum_out=sumsqc[ot][:, ch:ch + 1])

        def finalize_stats(k, work):
            """Column-reduce chunk partials, AllReduce across cores, compute
            BN affine s (scale) and t (shift) for block k."""
            nt = COUT[k] // 128
            sred = work.tile([128, 2 * nt], F32, tag="sred")
            for ot in range(nt):
                nc.vector.tensor_reduce(sred[:, ot:ot + 1], sumc[ot][:, :NCHUNK],
                                        axis=mybir.AxisListType.X, op=ALU.add)
                nc.vector.tensor_reduce(sred[:, nt + ot:nt + ot + 1],
                                        sumsqc[ot][:, :NCHUNK],
                                        axis=mybir.AxisListType.X, op=ALU.add)
            nc.gpsimd.dma_start(ar_in[k][:], sred[:])
            if os.environ.get("ADJ_NO_COLLECTIVE"):
                nc.gpsimd.dma_start(ar_out[k][:], ar_in[k][:])
            else:
                nc.gpsimd.collective_compute(
                    "AllReduce", ALU.add, replica_groups=[list(range(N_CORES))],
                    ins=[ar_in[k][:].opt()],
                    outs=[ar_out[k][:].opt()])
            gst = work.tile([128, 2 * nt], F32, tag="gst")
            nc.gpsimd.dma_start(gst[:], ar_out[k][:])
            mean = work.tile([128, nt], F32, tag="bn_mean")
            ey2 = work.tile([128, nt], F32, tag="bn_ey2")
            var = work.tile([128, nt], F32, tag="bn_var")
            sd = work.tile([128, nt], F32, tag="bn_sd")
            rd = work.tile([128, nt], F32, tag="bn_rd")
            tmp = work.tile([128, nt], F32, tag="bn_tmp")
            inv_n = 1.0 / float(NTOT)
            nc.vector.tensor_scalar_mul(mean[:], gst[:, 0:nt], inv_n)
            nc.vector.tensor_scalar_mul(ey2[:], gst[:, nt:2 * nt], inv_n)
            nc.vector.tensor_tensor(var[:], mean[:], mean[:], op=ALU.mult)
            nc.vector.tensor_tensor(var[:], ey2[:], var[:], op=ALU.subtract)
            nc.vector.tensor_scalar_add(var[:], var[:], EPS)
            nc.scalar.activation(sd[:], var[:], AF.Sqrt)
            nc.vector.reciprocal(rd[:], sd[:])
            g_ap = psb[k][:, 0:nt]
            be_ap = psb[k][:, nt:2 * nt]
            b_ap = psb[k][:, 2 * nt:3 * nt]
            nc.vector.tensor_tensor(s_sb[k][:], g_ap, rd[:], op=ALU.mult)
            # t = be - mean * s   (conv bias cancels inside batch-norm)
            nc.vector.tensor_tensor(tmp[:], mean[:], s_sb[k][:], op=ALU.mult)
            nc.vector.tensor_tensor(t_sb[k][:], be_ap, tmp[:], op=ALU.subtract)

        with tc.tile_pool(name="work", bufs=2) as work:
            # ================= phase 0: T = |x_i - x_j| -> conv0 -> y0 =====
            if trace_scopes:
                sc = nc.enter_named_scope("phase0")
            for ch in range(NCHUNK):
                n0, npx = chunk_pixels(ch)
                rows = [2 * ch, 2 * ch + 1][: (npx + V - 1) // V]
                tt = work.tile([128, 2 * CHUNK], F32, tag="tt")
                for ct in range(2):
                    for si, i in enumerate(rows):
                        nc.vector.tensor_scalar_sub(
                            tt[:, ct * CHUNK + si * V: ct * CHUNK + (si + 1) * V],
                            xT[:, ct * V:(ct + 1) * V],
                            xT[:, ct * V + i: ct * V + i + 1])
                for ct in range(2):
                    seg = tt[:, ct * CHUNK: ct * CHUNK + npx]
                    nc.scalar.activation(seg, seg, AF.Abs)
                ps = [psum.tile([128, CHUNK], F32, tag="ps", name="ps") for _ in range(2)]
                for ot in range(2):
                    for ct in range(2):
                        nc.tensor.matmul(
                            ps[ot][:, :npx],
                            wsb[0][:, ct * 256 + ot * 128: ct * 256 + (ot + 1) * 128],
                            tt[:, ct * CHUNK: ct * CHUNK + npx],
                            start=(ct == 0), stop=(ct == 1))
                stage = work.tile([128, 2 * CHUNK], F32, tag="stage")
                stats_and_store(0, ch, ps,
                                lambda ot: stage[:, ot * CHUNK: ot * CHUNK + npx],
                                work)
                for ot in range(2):
                    nc.sync.dma_start(ybuf[0][ot][:, n0:n0 + npx],
                                      stage[:, ot * CHUNK: ot * CHUNK + npx])
            finalize_stats(0, work)
            if trace_scopes:
                nc.leave_named_scope(sc)

            # ================= phases 1..3: conv blocks ====================
            for k in (1, 2, 3):
                if trace_scopes:
                    sc = nc.enter_named_scope(f"phase{k}")
                nti, nto = CIN[k] // 128, COUT[k] // 128
                for ch in range(NCHUNK):
                    n0, npx = chunk_pixels(ch)
                    # source of y_{k-1}
                    if k in (1, 2):
                        ysrc = work.tile([128, 2 * CHUNK], F32, tag="yin")
                        for ct in range(nti):
                            nc.sync.dma_start(
                                ysrc[:, ct * CHUNK: ct * CHUNK + npx],
                                ybuf[k - 1][ct][:, n0:n0 + npx])
                        src_ap = lambda ct: ysrc[:, ct * CHUNK: ct * CHUNK + npx]
                    else:
                        src_ap = lambda ct: bigbuf[:, n0:n0 + npx]
                    u = work.tile([128, 2 * CHUNK], F32, tag="u")
                    z = work.tile([128, 2 * CHUNK], F32, tag="z")
                    for ct in range(nti):
                        ua = u[:, ct * CHUNK: ct * CHUNK + npx]
                        za = z[:, ct * CHUNK: ct * CHUNK + npx]
                        nc.scalar.activation(ua, src_ap(ct), AF.Identity,
                                             bias=t_sb[k - 1][:, ct:ct + 1],
                                             scale=s_sb[k - 1][:, ct:ct + 1])
                        nc.vector.scalar_tensor_tensor(za, ua, SLOPE, ua,
                                                       op0=ALU.mult, op1=ALU.max)
                    ps = [psum.tile([128, CHUNK], F32, tag="ps", name="ps") for _ in range(nto)]
                    wk = wsb[k]
                    wct = COUT[k]  # columns per ct block in packed weight
                    for ot in range(nto):
                        for ct in range(nti):
                            nc.tensor.matmul(
                                ps[ot][:, :npx],
                                wk[:, ct * wct + ot * 128: ct * wct + (ot + 1) * 128],
                                z[:, ct * CHUNK: ct * CHUNK + npx],
                                start=(ct == 0), stop=(ct == nti - 1))
                    if k == 1:
                        stage = work.tile([128, 2 * CHUNK], F32, tag="stage")
                        stats_and_store(k, ch, ps,
                                        lambda ot: stage[:, ot * CHUNK: ot * CHUNK + npx],
                                        work)
                        for ot in range(nto):
                            nc.sync.dma_start(ybuf[1][ot][:, n0:n0 + npx],
                                              stage[:, ot * CHUNK: ot * CHUNK + npx])
                    else:
                        stats_and_store(k, ch, ps,
                                        lambda ot: bigbuf[:, n0:n0 + npx], work)
                finalize_stats(k, work)
                if trace_scopes:
                    nc.leave_named_scope(sc)

            # ================= phase 4: y3 -> logits =======================
            if trace_scopes:
                sc = nc.enter_named_scope("phase4")
            lg_flat = logits_d[:].rearrange("a b -> (a b)")
            for ch in range(NCHUNK):
                n0, npx = chunk_pixels(ch)
                u = work.tile([128, CHUNK], F32, tag="u", name="u")
                z = work.tile([128, CHUNK], F32, tag="z", name="z")
                nc.scalar.activation(u[:, :npx], bigbuf[:, n0:n0 + npx], AF.Identity,
                                     bias=t_sb[3][:, 0:1], scale=s_sb[3][:, 0:1])
                nc.vector.scalar_tensor_tensor(z[:, :npx], u[:, :npx], SLOPE,
                                               u[:, :npx], op0=ALU.mult, op1=ALU.max)
                lp = psum1.tile([1, CHUNK], F32, tag="lp")
                nc.tensor.matmul(lp[0:1, :npx], w4sb[:, 0:1], z[:, :npx],
                                 start=True, stop=True)
                lst = work.tile([1, CHUNK], F32, tag="stage", name="lst")
                nc.scalar.activation(lst[0:1, :npx], lp[0:1, :npx], AF.Copy)
                nc.sync.dma_start(lg_flat[n0:n0 + npx], lst[0:1, :npx])
            if trace_scopes:
                nc.leave_named_scope(sc)

        # ================= phase 5: softmax + topk mask ====================
        if trace_scopes:
            sc = nc.enter_named_scope("phase5")
        NR = (K // 8) + 1  # 13 max8 rounds to reach rank 100
        with tc.tile_pool(name="smax", bufs=2) as smax:
            for rt, (r0, nr) in enumerate([(0, 128), (128, V - 128)]):
                lt = smax.tile([128, V], F32, tag="lt")
                nc.sync.dma_start(lt[:nr, :], logits_d[r0:r0 + nr, :])
                lc = smax.tile([128, V], F32, tag="lc")
                nc.vector.tensor_copy(lc[:nr, :], lt[:nr, :])
                mx = smax.tile([128, 8 * NR], F32, tag="mx")
                for r in range(NR):
                    nc.vector.max(mx[:nr, 8 * r: 8 * (r + 1)], lc[:nr, :])
                    if r < NR - 1:
                        nc.vector.match_replace(lc[:nr, :],
                                                mx[:nr, 8 * r: 8 * (r + 1)],
                                                lc[:nr, :], -1e30)
                nmx = smax.tile([128, 1], F32, tag="nmx")
                nc.vector.tensor_scalar_mul(nmx[:nr, :], mx[:nr, 0:1], -1.0)
                et = smax.tile([128, V], F32, tag="et")
                rsum = smax.tile([128, 1], F32, tag="rsum")
                nc.scalar.activation(et[:nr, :], lt[:nr, :], AF.Exp,
                                     bias=nmx[:nr, 0:1], scale=1.0,
                                     accum_out=rsum[:nr, 0:1])
                rec = smax.tile([128, 1], F32, tag="rec")
                nc.vector.reciprocal(rec[:nr, :], rsum[:nr, :])
                pt = smax.tile([128, V], F32, tag="pt")
                nc.vector.tensor_scalar_mul(pt[:nr, :], et[:nr, :], rec[:nr, 0:1])
                ot_ = smax.tile([128, V], F32, tag="ot")
                nc.vector.scalar_tensor_tensor(ot_[:nr, :], lt[:nr, :],
                                               mx[:nr, K - 1:K], pt[:nr, :],
                                               op0=ALU.is_ge, op1=ALU.mult)
                nc.sync.dma_start(out_d[r0:r0 + nr, :], ot_[:nr, :])
        if trace_scopes:
            nc.leave_named_scope(sc)

    nc.finalize()
    return nc


def _prep_inputs(inputs):
    """Host-side reshape/transpose of the full inputs into per-core maps."""
    x = np.ascontiguousarray(inputs["x"], dtype=np.float32)

    def ctile(w):  # [cout, cin] -> [128, cin/128 * cout] packed per cin-tile
        wT = np.ascontiguousarray(w.T, dtype=np.float32)       # [cin, cout]
        cin, cout = wT.shape
        return np.ascontiguousarray(
            wT.reshape(cin // 128, 128, cout).transpose(1, 0, 2).reshape(128, -1))

    shared = {
        "w0t": ctile(inputs["w0"]), "w1t": ctile(inputs["w1"]),
        "w2t": ctile(inputs["w2"]), "w3t": ctile(inputs["w3"]),
        "w4t": ctile(inputs["w4"]),
    }

    def pcols(v):  # [cout] -> [128, cout/128]
        return np.ascontiguousarray(
            np.asarray(v, np.float32).reshape(-1, 128).T)

    for k in range(4):
        shared[f"p{k}"] = np.ascontiguousarray(np.concatenate(
            [pcols(inputs[f"g{k}"]), pcols(inputs[f"be{k}"]),
             pcols(inputs[f"b{k}"])], axis=1))

    in_maps = []
    for c in range(N_CORES):
        xt = np.ascontiguousarray(
            x[c].T.reshape(2, 128, V).transpose(1, 0, 2).reshape(128, 2 * V))
        in_maps.append({"xt": xt, **shared})
    return in_maps


_NC = None


def _get_nc():
    global _NC
    if _NC is None:
        _NC = _build_nc()
    return _NC


def kernel(**inputs):
    nc = _get_nc()
    in_maps = _prep_inputs(inputs)
    res = run_bass_kernel_spmd(nc, in_maps, core_ids=list(range(N_CORES)))
    return np.stack([res.results[c]["outb"] for c in range(N_CORES)], axis=0)
